# revision 40
# baseline (speedup 1.0000x reference)
"""DeepseekV2 MLA attention forward — Trainium2 Bass kernel (8 NeuronCores).

Sharding: data-parallel over batch (2) x sequence-parallel over query rows
(4 panels of 512) = 8 cores. Each core computes, for its (batch, panel):
  - q path (q_a_proj -> rmsnorm -> q_b_proj) for its 512 query rows
  - kv path (kv_a_proj -> rmsnorm -> kv_b_proj) for the FULL key sequence
  - RoPE, full attention (16 heads) for its query rows, o_proj
Output panels are concatenated on the host; no cross-core communication.

All matmul operands are bf16 (fp32 PSUM accumulation): full PE rate incl.
the 64-partition rope matmuls, half the DMA/copy traffic.  Weights are
pre-transposed on the host into per-partition-contiguous [m][p][k][c]
blocks so every weight DMA is one large contiguous burst.  Intermediates
(qaT, ckT, kpe, qnope, qpe, oT) stay resident in SBUF.  The attention kb
loop is software-pipelined so the PE never waits on the Act-engine exp.
"""

import os
import numpy as np
import ml_dtypes

import concourse.bass as bass
import concourse.bacc as bacc
import concourse.mybir as mybir
import concourse.tile as tile
from concourse import bass_utils

B, S, HID = 2, 2048, 2048
NH = 16
QLR, KVLR = 1536, 512
DN, DR, DV = 128, 64, 128
DQK = DN + DR
SCALE = DQK ** -0.5
EPS = 1e-6
P = 128
NPANEL = 4
W = S // NPANEL            # 512 query rows per core
NCORES = B * NPANEL

F32 = mybir.dt.float32
F32R = mybir.dt.float32r
BF16 = mybir.dt.bfloat16
NPBF = ml_dtypes.bfloat16
EXP = mybir.ActivationFunctionType.Exp
SQRT = mybir.ActivationFunctionType.Sqrt
SQUARE = mybir.ActivationFunctionType.Square
COPY = mybir.ActivationFunctionType.Copy
MULT = mybir.AluOpType.mult
ADD = mybir.AluOpType.add

KB_HID = HID // P          # 16
KB_QLR = QLR // P          # 12
KB_CKV = KVLR // P         # 4
KB_S = S // P              # 16
MB_QLR = QLR // P          # 12
MB_KVA = 5                 # 4 ckv blocks + 1 (zero-padded) rope block
MB_NOPE = NH * DN // P     # 16
MB_PE = NH * DR // P       # 8
MB_HID = HID // P          # 16
NCH = S // W               # 4 column chunks of the full sequence

LAST_RESULT = None         # BassKernelResults of the most recent launch


def _emit(tc, t, with_mask):
    nc = tc.nc
    mm = nc.tensor.matmul
    from contextlib import ExitStack
    top = ExitStack()

    const = top.enter_context(tc.tile_pool(name="const", bufs=1))
    ones_col = const.tile([P, 1], BF16)
    nc.vector.memset(ones_col[:], 1.0)
    ones_row = const.tile([1, P], BF16)
    nc.vector.memset(ones_row[:], 1.0)
    eps1 = const.tile([1, 1], F32)
    nc.vector.memset(eps1[:], EPS)
    qa_ln = const.tile([P, MB_QLR], F32)
    nc.sync.dma_start(qa_ln[:], t["qa_ln_p"][:])
    kva_ln = const.tile([P, KB_CKV], F32)
    nc.sync.dma_start(kva_ln[:], t["kva_ln_p"][:])
    # host rotates the key axis per core so the query panel is chunk 0;
    # cos/sin tables are rotated identically, so the q-rope tables are
    # just the first W columns of the full-S tables.
    cos2f = const.tile([P, S], BF16)
    sin2sf = const.tile([P, S], BF16)
    cos2p = cos2f[:, :W]
    sin2sp = sin2sf[:, :W]

    # persistent SBUF intermediates (all bf16)
    persist = top.enter_context(tc.tile_pool(name="persist", bufs=1))
    qaT = persist.tile([P, MB_QLR, W], BF16)       # q_a output, normalized
    ckT = persist.tile([P, KB_CKV, S], BF16)       # compressed kv, normalized
    kpe2 = persist.tile([P, S], BF16)              # roped k_pe, duplicated 2x
    qnopeT = persist.tile([P, MB_NOPE, W], BF16)
    qpeT = persist.tile([P, MB_PE, W], BF16)       # roped q_pe
    oT = persist.tile([P, NH, W], BF16)            # attn out (pre-o_proj)

    # Weight pools for later phases sit BELOW the per-phase scratch pools
    # in the SBUF stack, so their prefetch DMAs never carry a write-after-
    # read hazard against the previous phase's scratch tiles.
    pcw = top.enter_context(tc.tile_pool(name="phC_w", bufs=2))
    pdw = top.enter_context(tc.tile_pool(name="phD_w", bufs=2))
    pdv = top.enter_context(tc.tile_pool(name="phD_v", bufs=1))
    pdk = top.enter_context(tc.tile_pool(name="phD_k", bufs=2))
    pew = top.enter_context(tc.tile_pool(name="phE_w", bufs=2))

    # kv-path inputs: pools opened early, DMAs emitted inside phase A
    pbh = top.enter_context(tc.tile_pool(name="phB_h", bufs=2))
    wkva_pool = top.enter_context(tc.tile_pool(name="phB_w", bufs=1))
    wkva = wkva_pool.tile([P, MB_KVA, KB_HID, P], BF16)

    def rsqrt_bcast(pool, psum_pool, ss_ps, inv_dim):
        """[1,n] sum-of-squares psum -> [P,n] f32 PSUM of 1/sqrt(mean+eps).

        sqrt/square/copy share one Act table (sqrt_and_others), so
        phases A-C run with zero activation-table reloads.
        """
        n = ss_ps.shape[-1]
        srow = pool.tile([1, n], F32, tag="srow")
        nc.scalar.activation(srow[:], ss_ps[:], SQRT, bias=eps1[:],
                             scale=inv_dim)
        rrow = pool.tile([1, n], BF16, tag="rrow")
        with nc.allow_low_precision(reason="rmsnorm scale in bf16"):
            nc.vector.reciprocal(rrow[:], srow[:])
        bc_ps = psum_pool.tile([P, n], F32, tag="bcast")
        mm(bc_ps[:], ones_row[:], rrow[:], start=True, stop=True)
        return bc_ps

    # ------------- phase A: qaT panel + rmsnorm ----------------------
    # chunk 0 of the (rotated) sequence IS the query panel; load it into
    # the phase-B chunk pool and reuse it there without a second DMA.
    hn0 = pbh.tile([P, KB_HID, W], BF16, tag="hn")
    nc.sync.dma_start(hn0[:], t["hs_pks"][:, :, 0:W])
    with tc.tile_pool(name="phA", bufs=2) as pa, \
         tc.tile_pool(name="phA_w", bufs=2) as paw, \
         tc.tile_pool(name="psA", bufs=3, space="PSUM") as psA, \
         tc.tile_pool(name="psS", bufs=1, space="PSUM") as psSS, \
         tc.tile_pool(name="psB", bufs=1, space="PSUM") as psBC:
        ss = psSS.tile([1, W], F32, tag="ss")
        sq_prev = None
        for m in range(MB_QLR):
            wm = paw.tile([P, KB_HID, P], BF16, tag="wqa")
            nc.sync.dma_start(wm[:], t["w_qa"][m])
            if m == 1:
                # prefetch phase-B inputs off the critical path
                nc.sync.dma_start(wkva[:], t["w_kva"][:])
                nc.sync.dma_start(cos2f[:], t["cos2f"][:])
                nc.sync.dma_start(sin2sf[:], t["sin2sf"][:])
            ps = psA.tile([P, W], F32, tag="psA")
            for k in range(KB_HID):
                mm(ps[:], wm[:, k, :], hn0[:, k, :],
                   start=(k == 0), stop=(k == KB_HID - 1))
            nc.scalar.activation(qaT[:, m, :], ps[:], COPY)
            sq = pa.tile([P, W], BF16, tag="sq")
            nc.scalar.activation(sq[:], ps[:], SQUARE)
            if sq_prev is not None:
                mm(ss[:], ones_col[:], sq_prev,
                   start=(m == 1), stop=False, skip_group_check=True)
            sq_prev = sq[:]
        mm(ss[:], ones_col[:], sq_prev, start=False, stop=True,
           skip_group_check=True)
        rq = rsqrt_bcast(pa, psBC, ss[:], 1.0 / QLR)
        for m in range(MB_QLR):
            nc.vector.scalar_tensor_tensor(
                qaT[:, m, :], qaT[:, m, :], qa_ln[:, m:m + 1], rq[:],
                MULT, MULT)
        del rq

    # ------------- phase B: ckT (full S) + rmsnorm + kpe rope --------
    with tc.tile_pool(name="phB", bufs=2) as pb, \
         tc.tile_pool(name="psA", bufs=3, space="PSUM") as psA, \
         tc.tile_pool(name="psS", bufs=2, space="PSUM") as psSS, \
         tc.tile_pool(name="psB", bufs=2, space="PSUM") as psBC:
        for nch in range(NCH):
            if nch == 0:
                hn = hn0
            else:
                hn = pbh.tile([P, KB_HID, W], BF16, tag="hn")
                nc.sync.dma_start(
                    hn[:], t["hs_pks"][:, :, nch * W:(nch + 1) * W])
            ss = psSS.tile([1, W], F32, tag="ss")
            kp = pb.tile([P, W], BF16, tag="kp")
            sq_prev = None
            for m in range(MB_KVA):
                ps = psA.tile([P, W], F32, tag="psA")
                for k in range(KB_HID):
                    mm(ps[:], wkva[:, m, k, :], hn[:, k, :],
                       start=(k == 0), stop=(k == KB_HID - 1))
                if m < KB_CKV:
                    ckslc = ckT[:, m, nch * W:(nch + 1) * W]
                    nc.scalar.activation(ckslc, ps[:], COPY)
                    sq = pb.tile([P, W], BF16, tag="sq")
                    nc.scalar.activation(sq[:], ps[:], SQUARE)
                    if sq_prev is not None:
                        mm(ss[:], ones_col[:], sq_prev,
                           start=(m == 1), stop=False, skip_group_check=True)
                    sq_prev = sq[:]
                else:
                    mm(ss[:], ones_col[:], sq_prev, start=False, stop=True,
                       skip_group_check=True)
                    nc.scalar.activation(kp[0:DR, :], ps[0:DR, :], COPY)
                    nc.vector.tensor_copy(kp[DR:P, :], ps[0:DR, :])
            rk = rsqrt_bcast(pb, psBC, ss[:], 1.0 / KVLR)
            for m in range(KB_CKV):
                nc.vector.scalar_tensor_tensor(
                    ckT[:, m, nch * W:(nch + 1) * W],
                    ckT[:, m, nch * W:(nch + 1) * W],
                    kva_ln[:, m:m + 1], rk[:], MULT, MULT)
            del rk
            # RoPE on kp (both 64-halves hold the same data)
            rot = pb.tile([P, W], BF16, tag="rot")
            for h in (0, DR):
                nc.vector.tensor_copy(rot[h:h + 32, :], kp[h + 32:h + 64, :])
                nc.vector.tensor_copy(rot[h + 32:h + 64, :], kp[h:h + 32, :])
            csl = slice(nch * W, (nch + 1) * W)
            nc.vector.tensor_tensor(kp[:], kp[:], cos2f[:, csl], MULT)
            nc.vector.tensor_tensor(rot[:], rot[:], sin2sf[:, csl], MULT)
            nc.vector.tensor_tensor(kpe2[:, csl], kp[:], rot[:], ADD)

    # ------------- phase C: q_b panel (+ RoPE on pe part) ------------
    with tc.tile_pool(name="phC", bufs=2) as pc, \
         tc.tile_pool(name="psA", bufs=3, space="PSUM") as psA:
        for m in range(MB_NOPE + MB_PE):
            wm = pcw.tile([P, KB_QLR, P], BF16, tag="wqb")
            nc.sync.dma_start(wm[:], t["w_qb"][m])
            ps = psA.tile([P, W], F32, tag="psA")
            for k in range(KB_QLR):
                mm(ps[:], wm[:, k, :], qaT[:, k, :],
                   start=(k == 0), stop=(k == KB_QLR - 1))
            if m < MB_NOPE:
                nc.scalar.activation(qnopeT[:, m, :], ps[:], COPY)
            else:
                j = m - MB_NOPE
                qb = pc.tile([P, W], BF16, tag="qb")
                nc.scalar.activation(qb[:], ps[:], COPY)
                rotq = pc.tile([P, W], BF16, tag="rotq")
                for h in (0, DR):
                    nc.vector.tensor_copy(rotq[h:h + 32, :],
                                          qb[h + 32:h + 64, :])
                    nc.vector.tensor_copy(rotq[h + 32:h + 64, :],
                                          qb[h:h + 32, :])
                nc.vector.tensor_tensor(rotq[:], rotq[:], sin2sp[:], MULT)
                nc.vector.tensor_tensor(qpeT[:, j, :], qb[:], cos2p[:], MULT)
                nc.vector.tensor_tensor(qpeT[:, j, :], qpeT[:, j, :],
                                        rotq[:], ADD)

    # ------------- phase D: per 2-head group: V, knope, attention ----
    with tc.tile_pool(name="phD", bufs=2) as pd, \
         tc.tile_pool(name="probs", bufs=4) as pprob, \
         tc.tile_pool(name="pracc", bufs=2) as pracc_pool, \
         tc.tile_pool(name="psSc", bufs=4, space="PSUM") as psSc, \
         tc.tile_pool(name="psO", bufs=2, space="PSUM") as psO, \
         tc.tile_pool(name="psR", bufs=1, space="PSUM") as psR, \
         tc.tile_pool(name="psB2", bufs=1, space="PSUM") as psB2:
        from contextlib import ExitStack
        dctx = ExitStack()
        mask_pool = None
        if with_mask:
            mask_pool = dctx.enter_context(tc.tile_pool(name="maskp", bufs=4))

        # deferred normalization finish of the previous head, emitted
        # late so its PE bcast / DVE reciprocal never stall the in-order
        # PE stream.  pracc [P,W] holds sum over kb of probs (DVE adds);
        # the cross-partition sum and broadcast go through the PE.
        def finish_head(h, po, pracc):
            pab = pd.tile([P, W], BF16, tag="pab")
            nc.scalar.activation(pab[:], pracc[:], COPY)
            pr = psR.tile([1, W], F32, tag="pr")
            mm(pr[:], ones_col[:], pab[:], start=True, stop=True)
            rrow = pd.tile([1, W], BF16, tag="rrow")
            with nc.allow_low_precision(reason="softmax denom in bf16"):
                nc.vector.reciprocal(rrow[:], pr[:])
            bc_ps = psB2.tile([P, W], F32, tag="bcd")
            mm(bc_ps[:], ones_row[:], rrow[:], start=True, stop=True)
            bci = pd.tile([P, W], F32, tag="bci")
            nc.scalar.activation(bci[:], bc_ps[:], COPY)
            nc.vector.tensor_tensor(oT[:, h, :], po[:], bci[:], MULT)

        pending = None
        for g in range(NH // 2):
            # V for the 2 heads of this group: [128k, kb, 2*128]
            wv = pdw.tile([P, KB_CKV, 2 * DV], BF16, tag="wv")
            nc.sync.dma_start(wv[:], t["w_kvb_v"][g])
            v_sb = pdv.tile([P, KB_S, 2 * DV], BF16, tag="v")
            for kb in range(KB_S):
                psv = psSc.tile([P, W], F32, tag="pss")
                for kc in range(KB_CKV):
                    mm(psv[:, :2 * DV], ckT[:, kc, kb * P:(kb + 1) * P],
                       wv[:, kc, :], start=(kc == 0), stop=(kc == KB_CKV - 1))
                nc.scalar.activation(v_sb[:, kb, :], psv[:, :2 * DV], COPY)

            for hl in range(2):
                h = g * 2 + hl
                # knopeT for head h: [128 d, kb, 128 k]
                wkn = pdw.tile([P, KB_CKV, DN], BF16, tag="wkn")
                nc.sync.dma_start(wkn[:], t["w_kvb_kn"][h])
                knT = pdk.tile([P, KB_S, P], BF16, tag="knT")
                for nch in range(NCH):
                    psk = psSc.tile([P, W], F32, tag="pss")
                    for kc in range(KB_CKV):
                        mm(psk[:], wkn[:, kc, :],
                           ckT[:, kc, nch * W:(nch + 1) * W],
                           start=(kc == 0), stop=(kc == KB_CKV - 1))
                    nc.scalar.activation(
                        knT[:, nch * (W // P):(nch + 1) * (W // P), :],
                        psk[:], COPY)

                if pending is not None:
                    finish_head(*pending)
                    pending = None

                # attention for head h, software-pipelined over kb; probs
                # are f32 so the row-sum accumulates on the (idle) DVE
                # instead of burning PE rows
                po = psO.tile([P, W], F32, tag="po")
                pracc = pracc_pool.tile([P, W], F32, tag="pa")
                hp64 = hl * DR
                probs_q = []
                for kb in range(KB_S):
                    pss = psSc.tile([P, W], F32, tag="pss")
                    mm(pss[:], knT[:, kb, :], qnopeT[:, h, :],
                       start=True, stop=False)
                    mm(pss[:], kpe2[hp64:hp64 + DR, kb * P:(kb + 1) * P],
                       qpeT[hp64:hp64 + DR, g, :], start=False, stop=True)
                    probs = pprob.tile([P, W], BF16, tag="probs")
                    if with_mask:
                        mtile = mask_pool.tile([P, W], F32, tag="mt")
                        nc.sync.dma_start(
                            mtile[:], t["maskT"][kb * P:(kb + 1) * P, :])
                        pf = pprob.tile([P, W], F32, tag="probs_f")
                        nc.vector.scalar_tensor_tensor(
                            pf[:], pss[:], SCALE, mtile[:], MULT, ADD)
                        nc.scalar.activation(probs[:], pf[:], EXP)
                    else:
                        nc.scalar.activation(probs[:], pss[:], EXP,
                                             scale=SCALE)
                    if kb == 0:
                        nc.vector.tensor_copy(pracc[:], probs[:])
                    else:
                        nc.vector.tensor_tensor(pracc[:], pracc[:],
                                                probs[:], ADD)
                    probs_q.append((kb, probs))
                    if len(probs_q) == 4 or kb == KB_S - 1:
                        for kb2, pb2 in probs_q:
                            mm(po[:], v_sb[:, kb2, hl * DV:(hl + 1) * DV],
                               pb2[:], start=(kb2 == 0),
                               stop=(kb2 == KB_S - 1), skip_group_check=True)
                        probs_q = []
                pending = (h, po, pracc)
        finish_head(*pending)
        dctx.close()

    # ------------- phase E: o_proj -----------------------------------
    with tc.tile_pool(name="phE", bufs=2) as pe, \
         tc.tile_pool(name="psA", bufs=3, space="PSUM") as psA:
        for m in range(MB_HID):
            wm = pew.tile([P, NH, P], BF16, tag="wo")
            nc.sync.dma_start(wm[:], t["w_o"][m])
            ps = psA.tile([P, W], F32, tag="psA")
            for k in range(NH):
                mm(ps[:], wm[:, k, :], oT[:, k, :],
                   start=(k == 0), stop=(k == NH - 1))
            osb = pe.tile([P, W], F32, tag="osb")
            nc.scalar.activation(osb[:], ps[:], COPY)
            nc.sync.dma_start(t["outT"][m * P:(m + 1) * P, :], osb[:])
    top.close()


def _build_program(with_mask):
    nc = bacc.Bacc("TRN2", target_bir_lowering=False, debug=False)
    t = {}

    def inp(name, shape, dt=BF16):
        t[name] = nc.dram_tensor(name, list(shape), dt,
                                 kind="ExternalInput").ap()

    inp("hs_pks", [P, KB_HID, S])
    inp("w_qa", [MB_QLR, P, KB_HID, P])
    inp("w_qb", [MB_NOPE + MB_PE, P, KB_QLR, P])
    inp("w_kva", [P, MB_KVA, KB_HID, P])
    inp("w_kvb_kn", [NH, P, KB_CKV, DN])
    inp("w_kvb_v", [NH // 2, P, KB_CKV, 2 * DV])
    inp("w_o", [MB_HID, P, NH, P])
    inp("qa_ln_p", [P, MB_QLR], F32)
    inp("kva_ln_p", [P, KB_CKV], F32)
    inp("cos2f", [P, S])
    inp("sin2sf", [P, S])
    if with_mask:
        inp("maskT", [S, W], F32)
    t["outT"] = nc.dram_tensor("outT", [HID, W], F32,
                               kind="ExternalOutput").ap()

    with tile.TileContext(nc) as tc:
        _emit(tc, t, with_mask)
    nc.compile()
    return nc


_PROG_CACHE = {}


def _get_program(with_mask):
    if with_mask not in _PROG_CACHE:
        _PROG_CACHE[with_mask] = _build_program(with_mask)
    return _PROG_CACHE[with_mask]


def _block4(w, mb, kb):
    """[kb*P, mb*P] -> [mb, P, kb, P] with W[m,p,k,c] = w[k*P+p, m*P+c]."""
    return np.ascontiguousarray(
        w.reshape(kb, P, mb, P).transpose(2, 1, 0, 3))


def make_in_maps(hidden_states, attention_mask, cos, sin, w_qa, qa_ln, w_qb,
                 w_kva, kva_ln, w_kvb, w_o, with_mask):
    f32 = np.float32
    c = np.ascontiguousarray

    w_qb_r = np.asarray(w_qb, f32).reshape(QLR, NH, DQK)
    w_qb_re = np.concatenate(
        [w_qb_r[:, :, :DN].reshape(QLR, NH * DN),
         w_qb_r[:, :, DN:].reshape(QLR, NH * DR)], axis=1)
    w_kva_pad = np.concatenate(
        [np.asarray(w_kva, f32), np.zeros((HID, P - DR), f32)], axis=1)
    kvb = np.asarray(w_kvb, f32).reshape(KB_CKV, P, NH, DN + DV)
    w_kvb_kn = c(kvb[:, :, :, :DN].transpose(2, 1, 0, 3)
                 .astype(NPBF))                        # [NH, P, KB_CKV, DN]
    w_kvb_v = c(kvb[:, :, :, DN:].reshape(KB_CKV, P, NH // 2, 2 * DV)
                .transpose(2, 1, 0, 3).astype(NPBF))   # [NH/2, P, kc, 256]

    qa_ln_p = c(np.asarray(qa_ln, f32).reshape(MB_QLR, P).T)
    kva_ln_p = c(np.asarray(kva_ln, f32).reshape(KB_CKV, P).T)

    cosT = np.asarray(cos, f32).T                      # [64, S]
    sinT = np.asarray(sin, f32).T
    sin_s = np.concatenate([-sinT[:DR // 2], sinT[DR // 2:]], axis=0)
    cos2 = c(np.concatenate([cosT, cosT], axis=0))     # [128, S]
    sin2s = c(np.concatenate([sin_s, sin_s], axis=0))

    shared = {
        "w_qa": _block4(np.asarray(w_qa, f32), MB_QLR, KB_HID).astype(NPBF),
        "w_qb": _block4(w_qb_re, MB_NOPE + MB_PE, KB_QLR).astype(NPBF),
        "w_kva": c(w_kva_pad.reshape(KB_HID, P, MB_KVA, P)
                   .transpose(1, 2, 0, 3).astype(NPBF)),
        "w_kvb_kn": w_kvb_kn,
        "w_kvb_v": w_kvb_v,
        "w_o": _block4(np.asarray(w_o, f32), MB_HID, KB_HID).astype(NPBF),
        "qa_ln_p": qa_ln_p,
        "kva_ln_p": kva_ln_p,
    }

    hs = np.asarray(hidden_states)
    am = np.asarray(attention_mask)
    in_maps = []
    for core in range(NCORES):
        b, pnl = divmod(core, NPANEL)
        q0 = pnl * W
        # rotate the key axis so this core's query panel is chunk 0;
        # softmax over keys is permutation-invariant as long as the
        # rope tables (and mask) are rotated identically
        hsT = np.roll(np.asarray(hs[b], f32).T, -q0, axis=1)   # [HID, S]
        hs_pks = c(hsT.reshape(KB_HID, P, S).transpose(1, 0, 2)
                   .astype(NPBF))                      # [128, 16, S]
        m = dict(shared)
        m["hs_pks"] = hs_pks
        m["cos2f"] = c(np.roll(cos2, -q0, axis=1).astype(NPBF))
        m["sin2sf"] = c(np.roll(sin2s, -q0, axis=1).astype(NPBF))
        if with_mask:
            m["maskT"] = c(np.roll(am[b, 0, q0:q0 + W, :].T.astype(f32),
                                   -q0, axis=0))
        in_maps.append(m)
    return in_maps


def kernel(hidden_states, attention_mask, cos, sin, w_qa, qa_ln, w_qb,
           w_kva, kva_ln, w_kvb, w_o):
    global LAST_RESULT
    with_mask = bool(np.any(np.asarray(attention_mask) != 0))
    nc = _get_program(with_mask)
    in_maps = make_in_maps(hidden_states, attention_mask, cos, sin, w_qa,
                           qa_ln, w_qb, w_kva, kva_ln, w_kvb, w_o, with_mask)
    trace = os.environ.get("KERNEL_TRACE", "0") == "1"
    res = bass_utils.run_bass_kernel_spmd(
        nc, in_maps, core_ids=list(range(NCORES)), trace=trace)
    LAST_RESULT = res

    out = np.empty((B, S, HID), np.float32)
    for core in range(NCORES):
        b, pnl = divmod(core, NPANEL)
        q0 = pnl * W
        out[b, q0:q0 + W, :] = res.results[core]["outT"].T
    return out


# revision 44
# speedup vs baseline: 1.2763x; 1.2763x over previous
"""DeepseekV2 MLA attention forward — Trainium2 Bass kernel (8 NeuronCores).

Sharding: data-parallel over batch (2) x sequence-parallel over query rows
(4 panels of 512) = 8 cores. Each core computes, for its (batch, panel):
  - q path (q_a_proj -> rmsnorm -> q_b_proj) for its 512 query rows
  - kv path (kv_a_proj -> rmsnorm -> kv_b_proj) for the FULL key sequence
  - RoPE, full attention (16 heads) for its query rows, o_proj
Output panels are concatenated on the host; no cross-core communication.

All matmul operands are bf16 (fp32 PSUM accumulation): full PE rate incl.
the 64-partition rope matmuls, half the DMA/copy traffic.  Weights are
pre-transposed on the host into per-partition-contiguous [m][p][k][c]
blocks so every weight DMA is one large contiguous burst.  Intermediates
(qaT, ckT, kpe, qnope, qpe, oT) stay resident in SBUF.  The attention kb
loop is software-pipelined so the PE never waits on the Act-engine exp.
"""

import os
import numpy as np
import ml_dtypes

import concourse.bass as bass
import concourse.bacc as bacc
import concourse.mybir as mybir
import concourse.tile as tile
from concourse import bass_utils

B, S, HID = 2, 2048, 2048
NH = 16
QLR, KVLR = 1536, 512
DN, DR, DV = 128, 64, 128
DQK = DN + DR
SCALE = DQK ** -0.5
EPS = 1e-6
P = 128
NPANEL = 4
W = S // NPANEL            # 512 query rows per core
NCORES = B * NPANEL

F32 = mybir.dt.float32
F32R = mybir.dt.float32r
BF16 = mybir.dt.bfloat16
NPBF = ml_dtypes.bfloat16
EXP = mybir.ActivationFunctionType.Exp
SQRT = mybir.ActivationFunctionType.Sqrt
SQUARE = mybir.ActivationFunctionType.Square
COPY = mybir.ActivationFunctionType.Copy
MULT = mybir.AluOpType.mult
ADD = mybir.AluOpType.add

KB_HID = HID // P          # 16
KB_QLR = QLR // P          # 12
KB_CKV = KVLR // P         # 4
KB_S = S // P              # 16
MB_QLR = QLR // P          # 12
MB_KVA = 5                 # 4 ckv blocks + 1 (zero-padded) rope block
MB_NOPE = NH * DN // P     # 16
MB_PE = NH * DR // P       # 8
MB_HID = HID // P          # 16
NCH = S // W               # 4 column chunks of the full sequence

LAST_RESULT = None         # BassKernelResults of the most recent launch


def _emit(tc, t, with_mask):
    nc = tc.nc
    mm = nc.tensor.matmul
    from contextlib import ExitStack
    top = ExitStack()

    const = top.enter_context(tc.tile_pool(name="const", bufs=1))
    ones_col = const.tile([P, 1], BF16)
    nc.vector.memset(ones_col[:], 1.0)
    ones_row = const.tile([1, P], BF16)
    nc.vector.memset(ones_row[:], 1.0)
    eps1 = const.tile([1, 1], F32)
    nc.vector.memset(eps1[:], EPS)
    qa_ln = const.tile([P, MB_QLR], F32)
    nc.sync.dma_start(qa_ln[:], t["qa_ln_p"][:])
    kva_ln = const.tile([P, KB_CKV], F32)
    nc.sync.dma_start(kva_ln[:], t["kva_ln_p"][:])
    # host rotates the key axis per core so the query panel is chunk 0;
    # cos/sin tables are rotated identically, so the q-rope tables are
    # just the first W columns of the full-S tables.
    cos2f = const.tile([P, S], BF16)
    sin2sf = const.tile([P, S], BF16)
    cos2p = cos2f[:, :W]
    sin2sp = sin2sf[:, :W]

    # persistent SBUF intermediates (all bf16)
    persist = top.enter_context(tc.tile_pool(name="persist", bufs=1))
    qaT = persist.tile([P, MB_QLR, W], BF16)       # q_a output, normalized
    ckT = persist.tile([P, KB_CKV, S], BF16)       # compressed kv, normalized
    kpe2 = persist.tile([P, S], BF16)              # roped k_pe, duplicated 2x
    qnopeT = persist.tile([P, MB_NOPE, W], BF16)
    qpeT = persist.tile([P, MB_PE, W], BF16)       # roped q_pe
    oT = persist.tile([P, NH, W], BF16)            # attn out (pre-o_proj)

    # Weight pools for later phases sit BELOW the per-phase scratch pools
    # in the SBUF stack, so their prefetch DMAs never carry a write-after-
    # read hazard against the previous phase's scratch tiles.
    pcw = top.enter_context(tc.tile_pool(name="phC_w", bufs=2))
    pdw = top.enter_context(tc.tile_pool(name="phD_w", bufs=2))
    pdv = top.enter_context(tc.tile_pool(name="phD_v", bufs=1))
    pdk = top.enter_context(tc.tile_pool(name="phD_k", bufs=2))
    pew = top.enter_context(tc.tile_pool(name="phE_w", bufs=2))

    # kv-path inputs: pools opened early, DMAs emitted inside phase A
    pbh = top.enter_context(tc.tile_pool(name="phB_h", bufs=2))
    wkva_pool = top.enter_context(tc.tile_pool(name="phB_w", bufs=1))
    wkva = wkva_pool.tile([P, MB_KVA, KB_HID, P], BF16)

    def rsqrt_bcast(pool, psum_pool, ss_ps, inv_dim):
        """[1,n] sum-of-squares psum -> [P,n] f32 PSUM of 1/sqrt(mean+eps).

        sqrt/square/copy share one Act table (sqrt_and_others), so
        phases A-C run with zero activation-table reloads.
        """
        n = ss_ps.shape[-1]
        srow = pool.tile([1, n], F32, tag="srow")
        nc.scalar.activation(srow[:], ss_ps[:], SQRT, bias=eps1[:],
                             scale=inv_dim)
        rrow = pool.tile([1, n], BF16, tag="rrow")
        with nc.allow_low_precision(reason="rmsnorm scale in bf16"):
            nc.vector.reciprocal(rrow[:], srow[:])
        bc_ps = psum_pool.tile([P, n], F32, tag="bcast")
        mm(bc_ps[:], ones_row[:], rrow[:], start=True, stop=True)
        return bc_ps

    # ------------- phase A: qaT panel + rmsnorm ----------------------
    # chunk 0 of the (rotated) sequence IS the query panel; load it into
    # the phase-B chunk pool and reuse it there without a second DMA.
    hn0 = pbh.tile([P, KB_HID, W], BF16, tag="hn")
    nc.sync.dma_start(hn0[:], t["hs_pks"][:, :, 0:W])
    with tc.tile_pool(name="phA", bufs=2) as pa, \
         tc.tile_pool(name="phA_w", bufs=2) as paw, \
         tc.tile_pool(name="psA", bufs=3, space="PSUM") as psA, \
         tc.tile_pool(name="psS", bufs=1, space="PSUM") as psSS, \
         tc.tile_pool(name="psB", bufs=1, space="PSUM") as psBC:
        ss = psSS.tile([1, W], F32, tag="ss")
        sq_prev = None
        for m in range(MB_QLR):
            wm = paw.tile([P, KB_HID, P], BF16, tag="wqa")
            nc.sync.dma_start(wm[:], t["w_qa"][m])
            if m == 1:
                # prefetch phase-B inputs off the critical path
                nc.sync.dma_start(wkva[:], t["w_kva"][:])
                nc.sync.dma_start(cos2f[:], t["cos2f"][:])
                nc.sync.dma_start(sin2sf[:], t["sin2sf"][:])
            ps = psA.tile([P, W], F32, tag="psA")
            for k in range(KB_HID):
                mm(ps[:], wm[:, k, :], hn0[:, k, :],
                   start=(k == 0), stop=(k == KB_HID - 1))
            nc.scalar.activation(qaT[:, m, :], ps[:], COPY)
            sq = pa.tile([P, W], BF16, tag="sq")
            nc.scalar.activation(sq[:], ps[:], SQUARE)
            if sq_prev is not None:
                mm(ss[:], ones_col[:], sq_prev,
                   start=(m == 1), stop=False, skip_group_check=True)
            sq_prev = sq[:]
        mm(ss[:], ones_col[:], sq_prev, start=False, stop=True,
           skip_group_check=True)
        rq = rsqrt_bcast(pa, psBC, ss[:], 1.0 / QLR)
        for m in range(MB_QLR):
            nc.vector.scalar_tensor_tensor(
                qaT[:, m, :], qaT[:, m, :], qa_ln[:, m:m + 1], rq[:],
                MULT, MULT)
        del rq

    # ------------- phase B: ckT (full S) + rmsnorm + kpe rope --------
    with tc.tile_pool(name="phB", bufs=2) as pb, \
         tc.tile_pool(name="psA", bufs=3, space="PSUM") as psA, \
         tc.tile_pool(name="psS", bufs=2, space="PSUM") as psSS, \
         tc.tile_pool(name="psB", bufs=2, space="PSUM") as psBC:
        for nch in range(NCH):
            if nch == 0:
                hn = hn0
            else:
                hn = pbh.tile([P, KB_HID, W], BF16, tag="hn")
                nc.sync.dma_start(
                    hn[:], t["hs_pks"][:, :, nch * W:(nch + 1) * W])
            ss = psSS.tile([1, W], F32, tag="ss")
            kp = pb.tile([P, W], BF16, tag="kp")
            sq_prev = None
            for m in range(MB_KVA):
                ps = psA.tile([P, W], F32, tag="psA")
                for k in range(KB_HID):
                    mm(ps[:], wkva[:, m, k, :], hn[:, k, :],
                       start=(k == 0), stop=(k == KB_HID - 1))
                if m < KB_CKV:
                    ckslc = ckT[:, m, nch * W:(nch + 1) * W]
                    nc.scalar.activation(ckslc, ps[:], COPY)
                    sq = pb.tile([P, W], BF16, tag="sq")
                    nc.scalar.activation(sq[:], ps[:], SQUARE)
                    if sq_prev is not None:
                        mm(ss[:], ones_col[:], sq_prev,
                           start=(m == 1), stop=False, skip_group_check=True)
                    sq_prev = sq[:]
                else:
                    mm(ss[:], ones_col[:], sq_prev, start=False, stop=True,
                       skip_group_check=True)
                    nc.scalar.activation(kp[0:DR, :], ps[0:DR, :], COPY)
                    nc.vector.tensor_copy(kp[DR:P, :], ps[0:DR, :])
            rk = rsqrt_bcast(pb, psBC, ss[:], 1.0 / KVLR)
            for m in range(KB_CKV):
                nc.vector.scalar_tensor_tensor(
                    ckT[:, m, nch * W:(nch + 1) * W],
                    ckT[:, m, nch * W:(nch + 1) * W],
                    kva_ln[:, m:m + 1], rk[:], MULT, MULT)
            del rk
            # RoPE on kp (both 64-halves hold the same data)
            rot = pb.tile([P, W], BF16, tag="rot")
            for h in (0, DR):
                nc.vector.tensor_copy(rot[h:h + 32, :], kp[h + 32:h + 64, :])
                nc.vector.tensor_copy(rot[h + 32:h + 64, :], kp[h:h + 32, :])
            csl = slice(nch * W, (nch + 1) * W)
            nc.vector.tensor_tensor(kp[:], kp[:], cos2f[:, csl], MULT)
            nc.vector.tensor_tensor(rot[:], rot[:], sin2sf[:, csl], MULT)
            nc.vector.tensor_tensor(kpe2[:, csl], kp[:], rot[:], ADD)

    # ------------- phase C: q_b panel (+ RoPE on pe part) ------------
    with tc.tile_pool(name="phC", bufs=2) as pc, \
         tc.tile_pool(name="psA", bufs=3, space="PSUM") as psA:
        for m in range(MB_NOPE + MB_PE):
            wm = pcw.tile([P, KB_QLR, P], BF16, tag="wqb")
            nc.sync.dma_start(wm[:], t["w_qb"][m])
            ps = psA.tile([P, W], F32, tag="psA")
            for k in range(KB_QLR):
                mm(ps[:], wm[:, k, :], qaT[:, k, :],
                   start=(k == 0), stop=(k == KB_QLR - 1))
            if m < MB_NOPE:
                nc.scalar.activation(qnopeT[:, m, :], ps[:], COPY)
            else:
                j = m - MB_NOPE
                qb = pc.tile([P, W], BF16, tag="qb")
                nc.scalar.activation(qb[:], ps[:], COPY)
                rotq = pc.tile([P, W], BF16, tag="rotq")
                for h in (0, DR):
                    nc.vector.tensor_copy(rotq[h:h + 32, :],
                                          qb[h + 32:h + 64, :])
                    nc.vector.tensor_copy(rotq[h + 32:h + 64, :],
                                          qb[h:h + 32, :])
                nc.vector.tensor_tensor(rotq[:], rotq[:], sin2sp[:], MULT)
                nc.vector.tensor_tensor(qpeT[:, j, :], qb[:], cos2p[:], MULT)
                nc.vector.tensor_tensor(qpeT[:, j, :], qpeT[:, j, :],
                                        rotq[:], ADD)

    # ------------- phase D: per 2-head group: V, knope, attention ----
    with tc.tile_pool(name="phD", bufs=2) as pd, \
         tc.tile_pool(name="probs", bufs=4) as pprob, \
         tc.tile_pool(name="psSc", bufs=4, space="PSUM") as psSc, \
         tc.tile_pool(name="psO", bufs=2, space="PSUM") as psO, \
         tc.tile_pool(name="psR", bufs=1, space="PSUM") as psR, \
         tc.tile_pool(name="psB2", bufs=1, space="PSUM") as psB2:
        from contextlib import ExitStack
        dctx = ExitStack()
        mask_pool = None
        if with_mask:
            mask_pool = dctx.enter_context(tc.tile_pool(name="maskp", bufs=4))

        # deferred normalization finish of the previous head, emitted
        # late so its PE bcast / DVE reciprocal never stall the in-order
        # PE stream
        def finish_head(h, po, pr):
            rrow = pd.tile([1, W], BF16, tag="rrow")
            with nc.allow_low_precision(reason="softmax denom in bf16"):
                nc.vector.reciprocal(rrow[:], pr[:])
            bc_ps = psB2.tile([P, W], F32, tag="bcd")
            mm(bc_ps[:], ones_row[:], rrow[:], start=True, stop=True)
            bci = pd.tile([P, W], F32, tag="bci")
            nc.scalar.activation(bci[:], bc_ps[:], COPY)
            nc.vector.tensor_tensor(oT[:, h, :], po[:], bci[:], MULT)

        pending = None
        for g in range(NH // 2):
            # V for the 2 heads of this group: [128k, kb, 2*128]
            wv = pdw.tile([P, KB_CKV, 2 * DV], BF16, tag="wv")
            nc.sync.dma_start(wv[:], t["w_kvb_v"][g])
            v_sb = pdv.tile([P, KB_S, 2 * DV], BF16, tag="v")
            for kb in range(KB_S):
                psv = psSc.tile([P, W], F32, tag="pss")
                for kc in range(KB_CKV):
                    mm(psv[:, :2 * DV], ckT[:, kc, kb * P:(kb + 1) * P],
                       wv[:, kc, :], start=(kc == 0), stop=(kc == KB_CKV - 1))
                nc.scalar.activation(v_sb[:, kb, :], psv[:, :2 * DV], COPY)

            for hl in range(2):
                h = g * 2 + hl
                # knopeT for head h: [128 d, kb, 128 k]
                wkn = pdw.tile([P, KB_CKV, DN], BF16, tag="wkn")
                nc.sync.dma_start(wkn[:], t["w_kvb_kn"][h])
                knT = pdk.tile([P, KB_S, P], BF16, tag="knT")
                for nch in range(NCH):
                    psk = psSc.tile([P, W], F32, tag="pss")
                    for kc in range(KB_CKV):
                        mm(psk[:], wkn[:, kc, :],
                           ckT[:, kc, nch * W:(nch + 1) * W],
                           start=(kc == 0), stop=(kc == KB_CKV - 1))
                    nc.scalar.activation(
                        knT[:, nch * (W // P):(nch + 1) * (W // P), :],
                        psk[:], COPY)

                if pending is not None:
                    finish_head(*pending)
                    pending = None

                # attention for head h, software-pipelined over kb
                po = psO.tile([P, W], F32, tag="po")
                pr = psR.tile([1, W], F32, tag="pr")
                hp64 = hl * DR
                probs_q = []
                for kb in range(KB_S):
                    pss = psSc.tile([P, W], F32, tag="pss")
                    mm(pss[:], knT[:, kb, :], qnopeT[:, h, :],
                       start=True, stop=False)
                    mm(pss[:], kpe2[hp64:hp64 + DR, kb * P:(kb + 1) * P],
                       qpeT[hp64:hp64 + DR, g, :], start=False, stop=True)
                    probs = pprob.tile([P, W], BF16, tag="probs")
                    if with_mask:
                        mtile = mask_pool.tile([P, W], F32, tag="mt")
                        nc.sync.dma_start(
                            mtile[:], t["maskT"][kb * P:(kb + 1) * P, :])
                        pf = pprob.tile([P, W], F32, tag="probs_f")
                        nc.vector.scalar_tensor_tensor(
                            pf[:], pss[:], SCALE, mtile[:], MULT, ADD)
                        nc.scalar.activation(probs[:], pf[:], EXP)
                    else:
                        nc.scalar.activation(probs[:], pss[:], EXP,
                                             scale=SCALE)
                    probs_q.append((kb, probs))
                    if len(probs_q) == 4 or kb == KB_S - 1:
                        for kb2, pb2 in probs_q:
                            mm(po[:], v_sb[:, kb2, hl * DV:(hl + 1) * DV],
                               pb2[:], start=(kb2 == 0),
                               stop=(kb2 == KB_S - 1), skip_group_check=True)
                            mm(pr[:], ones_col[:], pb2[:],
                               start=(kb2 == 0), stop=(kb2 == KB_S - 1),
                               skip_group_check=True)
                        probs_q = []
                pending = (h, po, pr)
        finish_head(*pending)
        dctx.close()

    # ------------- phase E: o_proj -----------------------------------
    with tc.tile_pool(name="phE", bufs=2) as pe, \
         tc.tile_pool(name="psA", bufs=3, space="PSUM") as psA:
        for m in range(MB_HID):
            wm = pew.tile([P, NH, P], BF16, tag="wo")
            nc.sync.dma_start(wm[:], t["w_o"][m])
            ps = psA.tile([P, W], F32, tag="psA")
            for k in range(NH):
                mm(ps[:], wm[:, k, :], oT[:, k, :],
                   start=(k == 0), stop=(k == NH - 1))
            osb = pe.tile([P, W], F32, tag="osb")
            nc.scalar.activation(osb[:], ps[:], COPY)
            nc.sync.dma_start(t["outT"][m * P:(m + 1) * P, :], osb[:])
    top.close()


def _build_program(with_mask):
    nc = bacc.Bacc("TRN2", target_bir_lowering=False, debug=False)
    t = {}

    def inp(name, shape, dt=BF16):
        t[name] = nc.dram_tensor(name, list(shape), dt,
                                 kind="ExternalInput").ap()

    inp("hs_pks", [P, KB_HID, S])
    inp("w_qa", [MB_QLR, P, KB_HID, P])
    inp("w_qb", [MB_NOPE + MB_PE, P, KB_QLR, P])
    inp("w_kva", [P, MB_KVA, KB_HID, P])
    inp("w_kvb_kn", [NH, P, KB_CKV, DN])
    inp("w_kvb_v", [NH // 2, P, KB_CKV, 2 * DV])
    inp("w_o", [MB_HID, P, NH, P])
    inp("qa_ln_p", [P, MB_QLR], F32)
    inp("kva_ln_p", [P, KB_CKV], F32)
    inp("cos2f", [P, S])
    inp("sin2sf", [P, S])
    if with_mask:
        inp("maskT", [S, W], F32)
    t["outT"] = nc.dram_tensor("outT", [HID, W], F32,
                               kind="ExternalOutput").ap()

    with tile.TileContext(nc) as tc:
        _emit(tc, t, with_mask)
    nc.compile()
    return nc


_PROG_CACHE = {}


def _get_program(with_mask):
    if with_mask not in _PROG_CACHE:
        _PROG_CACHE[with_mask] = _build_program(with_mask)
    return _PROG_CACHE[with_mask]


def _block4(w, mb, kb):
    """[kb*P, mb*P] -> [mb, P, kb, P] with W[m,p,k,c] = w[k*P+p, m*P+c]."""
    return np.ascontiguousarray(
        w.reshape(kb, P, mb, P).transpose(2, 1, 0, 3))


def make_in_maps(hidden_states, attention_mask, cos, sin, w_qa, qa_ln, w_qb,
                 w_kva, kva_ln, w_kvb, w_o, with_mask):
    f32 = np.float32
    c = np.ascontiguousarray

    w_qb_r = np.asarray(w_qb, f32).reshape(QLR, NH, DQK)
    w_qb_re = np.concatenate(
        [w_qb_r[:, :, :DN].reshape(QLR, NH * DN),
         w_qb_r[:, :, DN:].reshape(QLR, NH * DR)], axis=1)
    w_kva_pad = np.concatenate(
        [np.asarray(w_kva, f32), np.zeros((HID, P - DR), f32)], axis=1)
    kvb = np.asarray(w_kvb, f32).reshape(KB_CKV, P, NH, DN + DV)
    w_kvb_kn = c(kvb[:, :, :, :DN].transpose(2, 1, 0, 3)
                 .astype(NPBF))                        # [NH, P, KB_CKV, DN]
    w_kvb_v = c(kvb[:, :, :, DN:].reshape(KB_CKV, P, NH // 2, 2 * DV)
                .transpose(2, 1, 0, 3).astype(NPBF))   # [NH/2, P, kc, 256]

    qa_ln_p = c(np.asarray(qa_ln, f32).reshape(MB_QLR, P).T)
    kva_ln_p = c(np.asarray(kva_ln, f32).reshape(KB_CKV, P).T)

    cosT = np.asarray(cos, f32).T                      # [64, S]
    sinT = np.asarray(sin, f32).T
    sin_s = np.concatenate([-sinT[:DR // 2], sinT[DR // 2:]], axis=0)
    cos2 = c(np.concatenate([cosT, cosT], axis=0))     # [128, S]
    sin2s = c(np.concatenate([sin_s, sin_s], axis=0))

    shared = {
        "w_qa": _block4(np.asarray(w_qa, f32), MB_QLR, KB_HID).astype(NPBF),
        "w_qb": _block4(w_qb_re, MB_NOPE + MB_PE, KB_QLR).astype(NPBF),
        "w_kva": c(w_kva_pad.reshape(KB_HID, P, MB_KVA, P)
                   .transpose(1, 2, 0, 3).astype(NPBF)),
        "w_kvb_kn": w_kvb_kn,
        "w_kvb_v": w_kvb_v,
        "w_o": _block4(np.asarray(w_o, f32), MB_HID, KB_HID).astype(NPBF),
        "qa_ln_p": qa_ln_p,
        "kva_ln_p": kva_ln_p,
    }

    hs = np.asarray(hidden_states)
    am = np.asarray(attention_mask)
    in_maps = []
    for core in range(NCORES):
        b, pnl = divmod(core, NPANEL)
        q0 = pnl * W
        # rotate the key axis so this core's query panel is chunk 0;
        # softmax over keys is permutation-invariant as long as the
        # rope tables (and mask) are rotated identically
        hsT = np.roll(np.asarray(hs[b], f32).T, -q0, axis=1)   # [HID, S]
        hs_pks = c(hsT.reshape(KB_HID, P, S).transpose(1, 0, 2)
                   .astype(NPBF))                      # [128, 16, S]
        m = dict(shared)
        m["hs_pks"] = hs_pks
        m["cos2f"] = c(np.roll(cos2, -q0, axis=1).astype(NPBF))
        m["sin2sf"] = c(np.roll(sin2s, -q0, axis=1).astype(NPBF))
        if with_mask:
            m["maskT"] = c(np.roll(am[b, 0, q0:q0 + W, :].T.astype(f32),
                                   -q0, axis=0))
        in_maps.append(m)
    return in_maps


def kernel(hidden_states, attention_mask, cos, sin, w_qa, qa_ln, w_qb,
           w_kva, kva_ln, w_kvb, w_o):
    global LAST_RESULT
    with_mask = bool(np.any(np.asarray(attention_mask) != 0))
    nc = _get_program(with_mask)
    in_maps = make_in_maps(hidden_states, attention_mask, cos, sin, w_qa,
                           qa_ln, w_qb, w_kva, kva_ln, w_kvb, w_o, with_mask)
    trace = os.environ.get("KERNEL_TRACE", "0") == "1"
    res = bass_utils.run_bass_kernel_spmd(
        nc, in_maps, core_ids=list(range(NCORES)), trace=trace)
    LAST_RESULT = res

    out = np.empty((B, S, HID), np.float32)
    for core in range(NCORES):
        b, pnl = divmod(core, NPANEL)
        q0 = pnl * W
        out[b, q0:q0 + W, :] = res.results[core]["outT"].T
    return out


# revision 52
# speedup vs baseline: 1.3501x; 1.0579x over previous
"""DeepseekV2 MLA attention forward — Trainium2 Bass kernel (8 NeuronCores).

Sharding: data-parallel over batch (2) x sequence-parallel over query rows
(4 panels of 512) = 8 cores. Each core computes, for its (batch, panel):
  - q path (q_a_proj -> rmsnorm -> q_b_proj) for its 512 query rows
  - kv path (kv_a_proj -> rmsnorm -> kv_b_proj) for the FULL key sequence
  - RoPE, full attention (16 heads) for its query rows, o_proj
Output panels are concatenated on the host; no cross-core communication.

All matmul operands are bf16 (fp32 PSUM accumulation): full PE rate incl.
the 64-partition rope matmuls, half the DMA/copy traffic.  Weights are
pre-transposed on the host into per-partition-contiguous [m][p][k][c]
blocks so every weight DMA is one large contiguous burst.  Intermediates
(qaT, ckT, kpe, qnope, qpe, oT) stay resident in SBUF.  The attention kb
loop is software-pipelined so the PE never waits on the Act-engine exp.
"""

import os
import numpy as np
import ml_dtypes

import concourse.bass as bass
import concourse.bacc as bacc
import concourse.mybir as mybir
import concourse.tile as tile
from concourse import bass_utils

B, S, HID = 2, 2048, 2048
NH = 16
QLR, KVLR = 1536, 512
DN, DR, DV = 128, 64, 128
DQK = DN + DR
SCALE = DQK ** -0.5
EPS = 1e-6
P = 128
NPANEL = 4
W = S // NPANEL            # 512 query rows per core
NCORES = B * NPANEL

F32 = mybir.dt.float32
F32R = mybir.dt.float32r
BF16 = mybir.dt.bfloat16
NPBF = ml_dtypes.bfloat16
EXP = mybir.ActivationFunctionType.Exp
SQRT = mybir.ActivationFunctionType.Sqrt
SQUARE = mybir.ActivationFunctionType.Square
COPY = mybir.ActivationFunctionType.Copy
MULT = mybir.AluOpType.mult
ADD = mybir.AluOpType.add

KB_HID = HID // P          # 16
KB_QLR = QLR // P          # 12
KB_CKV = KVLR // P         # 4
KB_S = S // P              # 16
MB_QLR = QLR // P          # 12
MB_KVA = 5                 # 4 ckv blocks + 1 (zero-padded) rope block
MB_NOPE = NH * DN // P     # 16
MB_PE = NH * DR // P       # 8
MB_HID = HID // P          # 16
NCH = S // W               # 4 column chunks of the full sequence

LAST_RESULT = None         # BassKernelResults of the most recent launch


def _emit(tc, t, with_mask):
    nc = tc.nc
    mm = nc.tensor.matmul
    from contextlib import ExitStack
    top = ExitStack()

    const = top.enter_context(tc.tile_pool(name="const", bufs=1))
    ones_col = const.tile([P, 1], BF16)
    nc.vector.memset(ones_col[:], 1.0)
    ones_row = const.tile([1, P], BF16)
    nc.vector.memset(ones_row[:], 1.0)

    eps1 = const.tile([1, 1], F32)
    nc.vector.memset(eps1[:], EPS)
    qa_ln = const.tile([P, MB_QLR], F32)
    nc.sync.dma_start(qa_ln[:], t["qa_ln_p"][:])
    kva_ln = const.tile([P, KB_CKV], F32)
    nc.sync.dma_start(kva_ln[:], t["kva_ln_p"][:])
    # host rotates the key axis per core so the query panel is chunk 0;
    # cos/sin tables are rotated identically, so the q-rope tables are
    # just the first W columns of the full-S tables.
    cos2f = const.tile([P, S], BF16)
    sin2sf = const.tile([P, S], BF16)
    cos2p = cos2f[:, :W]
    sin2sp = sin2sf[:, :W]

    # persistent SBUF intermediates (all bf16)
    persist = top.enter_context(tc.tile_pool(name="persist", bufs=1))
    qaT = persist.tile([P, MB_QLR, W], BF16)       # q_a output, normalized
    ckT = persist.tile([P, KB_CKV, S], BF16)       # compressed kv, normalized
    kpe2 = persist.tile([P, S], BF16)              # roped k_pe, duplicated 2x
    qnopeT = persist.tile([P, MB_NOPE, W], BF16)
    qpeT = persist.tile([P, MB_PE, W], BF16)       # roped q_pe
    oT = persist.tile([P, NH, W], BF16)            # attn out (pre-o_proj)

    # Weight pools for later phases sit BELOW the per-phase scratch pools
    # in the SBUF stack, so their prefetch DMAs never carry a write-after-
    # read hazard against the previous phase's scratch tiles.
    pcw = top.enter_context(tc.tile_pool(name="phC_w", bufs=2))
    pdw = top.enter_context(tc.tile_pool(name="phD_w", bufs=2))
    pdv = top.enter_context(tc.tile_pool(name="phD_v", bufs=1))
    pdk = top.enter_context(tc.tile_pool(name="phD_k", bufs=2))
    pew = top.enter_context(tc.tile_pool(name="phE_w", bufs=2))

    # kv-path inputs: pools opened early, DMAs emitted inside phase A
    pbh = top.enter_context(tc.tile_pool(name="phB_h", bufs=2))
    wkva_pool = top.enter_context(tc.tile_pool(name="phB_w", bufs=1))
    wkva = wkva_pool.tile([P, MB_KVA, KB_HID, P], BF16)

    def rsqrt_bcast(pool, psum_pool, ss_ps, inv_dim):
        """[1,n] sum-of-squares psum -> [P,n] f32 PSUM of 1/sqrt(mean+eps).

        sqrt/square/copy share one Act table (sqrt_and_others), so
        phases A-C run with zero activation-table reloads.
        """
        n = ss_ps.shape[-1]
        srow = pool.tile([1, n], F32, tag="srow")
        nc.scalar.activation(srow[:], ss_ps[:], SQRT, bias=eps1[:],
                             scale=inv_dim)
        rrow = pool.tile([1, n], F32, tag="rrow")
        nc.vector.reciprocal_approx_fast(rrow[:], srow[:])
        rrow_bf = pool.tile([1, n], BF16, tag="rrow_bf")
        nc.scalar.activation(rrow_bf[:], rrow[:], COPY)
        bc_ps = psum_pool.tile([P, n], F32, tag="bcast")
        mm(bc_ps[:], ones_row[:], rrow_bf[:], start=True, stop=True)
        return bc_ps

    # ------------- phase A: qaT panel + rmsnorm ----------------------
    # chunk 0 of the (rotated) sequence IS the query panel; load it into
    # the phase-B chunk pool and reuse it there without a second DMA.
    hn0 = pbh.tile([P, KB_HID, W], BF16, tag="hn")
    nc.sync.dma_start(hn0[:], t["hs_pks"][:, :, 0:W])
    with tc.tile_pool(name="phA", bufs=2) as pa, \
         tc.tile_pool(name="phA_w", bufs=2) as paw, \
         tc.tile_pool(name="psA", bufs=3, space="PSUM") as psA, \
         tc.tile_pool(name="psS", bufs=1, space="PSUM") as psSS, \
         tc.tile_pool(name="psB", bufs=1, space="PSUM") as psBC:
        ss = psSS.tile([1, W], F32, tag="ss")
        sq_prev = None
        for m in range(MB_QLR):
            wm = paw.tile([P, KB_HID, P], BF16, tag="wqa")
            nc.sync.dma_start(wm[:], t["w_qa"][m])
            if m == 1:
                # prefetch phase-B inputs off the critical path
                nc.sync.dma_start(wkva[:], t["w_kva"][:])
                nc.sync.dma_start(cos2f[:], t["cos2f"][:])
                nc.sync.dma_start(sin2sf[:], t["sin2sf"][:])
            ps = psA.tile([P, W], F32, tag="psA")
            for k in range(KB_HID):
                mm(ps[:], wm[:, k, :], hn0[:, k, :],
                   start=(k == 0), stop=(k == KB_HID - 1))
            nc.scalar.activation(qaT[:, m, :], ps[:], COPY)
            sq = pa.tile([P, W], BF16, tag="sq")
            nc.scalar.activation(sq[:], ps[:], SQUARE)
            if sq_prev is not None:
                mm(ss[:], ones_col[:], sq_prev,
                   start=(m == 1), stop=False, skip_group_check=True)
            sq_prev = sq[:]
        mm(ss[:], ones_col[:], sq_prev, start=False, stop=True,
           skip_group_check=True)
        rq = rsqrt_bcast(pa, psBC, ss[:], 1.0 / QLR)
        for m in range(MB_QLR):
            nc.vector.scalar_tensor_tensor(
                qaT[:, m, :], qaT[:, m, :], qa_ln[:, m:m + 1], rq[:],
                MULT, MULT)
        del rq

    # ------------- phase B: ckT (full S) + rmsnorm + kpe rope --------
    with tc.tile_pool(name="phB", bufs=2) as pb, \
         tc.tile_pool(name="psA", bufs=3, space="PSUM") as psA, \
         tc.tile_pool(name="psS", bufs=2, space="PSUM") as psSS, \
         tc.tile_pool(name="psB", bufs=2, space="PSUM") as psBC:
        for nch in range(NCH):
            if nch == 0:
                hn = hn0
            else:
                hn = pbh.tile([P, KB_HID, W], BF16, tag="hn")
                nc.sync.dma_start(
                    hn[:], t["hs_pks"][:, :, nch * W:(nch + 1) * W])
            ss = psSS.tile([1, W], F32, tag="ss")
            kp = pb.tile([P, W], BF16, tag="kp")
            sq_prev = None
            for m in range(MB_KVA):
                ps = psA.tile([P, W], F32, tag="psA")
                for k in range(KB_HID):
                    mm(ps[:], wkva[:, m, k, :], hn[:, k, :],
                       start=(k == 0), stop=(k == KB_HID - 1))
                if m < KB_CKV:
                    ckslc = ckT[:, m, nch * W:(nch + 1) * W]
                    nc.scalar.activation(ckslc, ps[:], COPY)
                    sq = pb.tile([P, W], BF16, tag="sq")
                    nc.scalar.activation(sq[:], ps[:], SQUARE)
                    if sq_prev is not None:
                        mm(ss[:], ones_col[:], sq_prev,
                           start=(m == 1), stop=False, skip_group_check=True)
                    sq_prev = sq[:]
                else:
                    mm(ss[:], ones_col[:], sq_prev, start=False, stop=True,
                       skip_group_check=True)
                    nc.scalar.activation(kp[0:DR, :], ps[0:DR, :], COPY)
                    nc.vector.tensor_copy(kp[DR:P, :], ps[0:DR, :])
            rk = rsqrt_bcast(pb, psBC, ss[:], 1.0 / KVLR)
            for m in range(KB_CKV):
                nc.vector.scalar_tensor_tensor(
                    ckT[:, m, nch * W:(nch + 1) * W],
                    ckT[:, m, nch * W:(nch + 1) * W],
                    kva_ln[:, m:m + 1], rk[:], MULT, MULT)
            del rk
            # RoPE on kp (both 64-halves hold the same data)
            rot = pb.tile([P, W], BF16, tag="rot")
            for h in (0, DR):
                nc.vector.tensor_copy(rot[h:h + 32, :], kp[h + 32:h + 64, :])
                nc.vector.tensor_copy(rot[h + 32:h + 64, :], kp[h:h + 32, :])
            csl = slice(nch * W, (nch + 1) * W)
            nc.vector.tensor_tensor(kp[:], kp[:], cos2f[:, csl], MULT)
            nc.vector.tensor_tensor(rot[:], rot[:], sin2sf[:, csl], MULT)
            nc.vector.tensor_tensor(kpe2[:, csl], kp[:], rot[:], ADD)

    # ------------- phase C: q_b panel (+ RoPE on pe part) ------------
    with tc.tile_pool(name="phC", bufs=2) as pc, \
         tc.tile_pool(name="psA", bufs=3, space="PSUM") as psA:
        for m in range(MB_NOPE + MB_PE):
            wm = pcw.tile([P, KB_QLR, P], BF16, tag="wqb")
            nc.sync.dma_start(wm[:], t["w_qb"][m])
            ps = psA.tile([P, W], F32, tag="psA")
            for k in range(KB_QLR):
                mm(ps[:], wm[:, k, :], qaT[:, k, :],
                   start=(k == 0), stop=(k == KB_QLR - 1))
            if m < MB_NOPE:
                nc.scalar.activation(qnopeT[:, m, :], ps[:], COPY)
            else:
                j = m - MB_NOPE
                qb = pc.tile([P, W], BF16, tag="qb")
                nc.scalar.activation(qb[:], ps[:], COPY)
                rotq = pc.tile([P, W], BF16, tag="rotq")
                for h in (0, DR):
                    nc.vector.tensor_copy(rotq[h:h + 32, :],
                                          qb[h + 32:h + 64, :])
                    nc.vector.tensor_copy(rotq[h + 32:h + 64, :],
                                          qb[h:h + 32, :])
                nc.vector.tensor_tensor(rotq[:], rotq[:], sin2sp[:], MULT)
                nc.vector.tensor_tensor(qpeT[:, j, :], qb[:], cos2p[:], MULT)
                nc.vector.tensor_tensor(qpeT[:, j, :], qpeT[:, j, :],
                                        rotq[:], ADD)

    # ------------- phase D: per 2-head group: V, knope, attention ----
    with tc.tile_pool(name="phD", bufs=2) as pd, \
         tc.tile_pool(name="probs", bufs=4) as pprob, \
         tc.tile_pool(name="psSc", bufs=4, space="PSUM") as psSc, \
         tc.tile_pool(name="psO", bufs=2, space="PSUM") as psO, \
         tc.tile_pool(name="psR", bufs=1, space="PSUM") as psR, \
         tc.tile_pool(name="psB2", bufs=1, space="PSUM") as psB2:
        from contextlib import ExitStack
        dctx = ExitStack()
        mask_pool = None
        if with_mask:
            mask_pool = dctx.enter_context(tc.tile_pool(name="maskp", bufs=4))

        # deferred normalization finish of the previous head, emitted
        # late so its PE bcast / DVE reciprocal never stall the in-order
        # PE stream
        def finish_head(h, po, pr):
            rrow = pd.tile([1, W], F32, tag="rrow")
            nc.vector.reciprocal_approx_fast(rrow[:], pr[:])
            rrow_bf = pd.tile([1, W], BF16, tag="rrow_bf")
            nc.scalar.activation(rrow_bf[:], rrow[:], COPY)
            bc_ps = psB2.tile([P, W], F32, tag="bcd")
            mm(bc_ps[:], ones_row[:], rrow_bf[:], start=True, stop=True)
            bci = pd.tile([P, W], F32, tag="bci")
            nc.scalar.activation(bci[:], bc_ps[:], COPY)
            nc.vector.tensor_tensor(oT[:, h, :], po[:], bci[:], MULT)

        pending = None
        for g in range(NH // 2):
            # V for the 2 heads of this group: [128k, kb, 2*128]
            wv = pdw.tile([P, KB_CKV, 2 * DV], BF16, tag="wv")
            nc.sync.dma_start(wv[:], t["w_kvb_v"][g])
            v_sb = pdv.tile([P, KB_S, 2 * DV], BF16, tag="v")
            for kb in range(KB_S):
                psv = psSc.tile([P, W], F32, tag="pss")
                for kc in range(KB_CKV):
                    mm(psv[:, :2 * DV], ckT[:, kc, kb * P:(kb + 1) * P],
                       wv[:, kc, :], start=(kc == 0), stop=(kc == KB_CKV - 1))
                nc.scalar.activation(v_sb[:, kb, :], psv[:, :2 * DV], COPY)

            for hl in range(2):
                h = g * 2 + hl
                # knopeT for head h: [128 d, kb, 128 k]
                wkn = pdw.tile([P, KB_CKV, DN], BF16, tag="wkn")
                nc.sync.dma_start(wkn[:], t["w_kvb_kn"][h])
                knT = pdk.tile([P, KB_S, P], BF16, tag="knT")
                for nch in range(NCH):
                    psk = psSc.tile([P, W], F32, tag="pss")
                    for kc in range(KB_CKV):
                        mm(psk[:], wkn[:, kc, :],
                           ckT[:, kc, nch * W:(nch + 1) * W],
                           start=(kc == 0), stop=(kc == KB_CKV - 1))
                    nc.scalar.activation(
                        knT[:, nch * (W // P):(nch + 1) * (W // P), :],
                        psk[:], COPY)

                # attention for head h, software-pipelined over kb
                po = psO.tile([P, W], F32, tag="po")
                pr = psR.tile([1, W], F32, tag="pr")
                hp64 = hl * DR
                probs_q = []
                for kb in range(KB_S):
                    pss = psSc.tile([P, W], F32, tag="pss")
                    mm(pss[:], knT[:, kb, :], qnopeT[:, h, :],
                       start=True, stop=False)
                    mm(pss[:], kpe2[hp64:hp64 + DR, kb * P:(kb + 1) * P],
                       qpeT[hp64:hp64 + DR, g, :], start=False, stop=True)
                    probs = pprob.tile([P, W], BF16, tag="probs")
                    if with_mask:
                        mtile = mask_pool.tile([P, W], F32, tag="mt")
                        nc.sync.dma_start(
                            mtile[:], t["maskT"][kb * P:(kb + 1) * P, :])
                        pf = pprob.tile([P, W], F32, tag="probs_f")
                        nc.vector.scalar_tensor_tensor(
                            pf[:], pss[:], SCALE, mtile[:], MULT, ADD)
                        nc.scalar.activation(probs[:], pf[:], EXP)
                    else:
                        nc.scalar.activation(probs[:], pss[:], EXP,
                                             scale=SCALE)
                    probs_q.append((kb, probs))
                    if len(probs_q) == 4 or kb == KB_S - 1:
                        for kb2, pb2 in probs_q:
                            mm(po[:], v_sb[:, kb2, hl * DV:(hl + 1) * DV],
                               pb2[:], start=(kb2 == 0),
                               stop=(kb2 == KB_S - 1), skip_group_check=True)
                            mm(pr[:], ones_col[:], pb2[:],
                               start=(kb2 == 0), stop=(kb2 == KB_S - 1),
                               skip_group_check=True)
                        probs_q = []
                        # finish the previous head once this head's first
                        # kb batch is in flight: its reciprocal has had
                        # time to drain, so the PE bcast never stalls
                        if pending is not None:
                            finish_head(*pending)
                            pending = None
                pending = (h, po, pr)
        finish_head(*pending)
        dctx.close()

    # ------------- phase E: o_proj -----------------------------------
    with tc.tile_pool(name="phE", bufs=2) as pe, \
         tc.tile_pool(name="psA", bufs=3, space="PSUM") as psA:
        for m in range(MB_HID):
            wm = pew.tile([P, NH, P], BF16, tag="wo")
            nc.sync.dma_start(wm[:], t["w_o"][m])
            ps = psA.tile([P, W], F32, tag="psA")
            for k in range(NH):
                mm(ps[:], wm[:, k, :], oT[:, k, :],
                   start=(k == 0), stop=(k == NH - 1))
            osb = pe.tile([P, W], F32, tag="osb")
            nc.scalar.activation(osb[:], ps[:], COPY)
            nc.sync.dma_start(t["outT"][m * P:(m + 1) * P, :], osb[:])
    top.close()


def _build_program(with_mask):
    nc = bacc.Bacc("TRN2", target_bir_lowering=False, debug=False)
    t = {}

    def inp(name, shape, dt=BF16):
        t[name] = nc.dram_tensor(name, list(shape), dt,
                                 kind="ExternalInput").ap()

    inp("hs_pks", [P, KB_HID, S])
    inp("w_qa", [MB_QLR, P, KB_HID, P])
    inp("w_qb", [MB_NOPE + MB_PE, P, KB_QLR, P])
    inp("w_kva", [P, MB_KVA, KB_HID, P])
    inp("w_kvb_kn", [NH, P, KB_CKV, DN])
    inp("w_kvb_v", [NH // 2, P, KB_CKV, 2 * DV])
    inp("w_o", [MB_HID, P, NH, P])
    inp("qa_ln_p", [P, MB_QLR], F32)
    inp("kva_ln_p", [P, KB_CKV], F32)
    inp("cos2f", [P, S])
    inp("sin2sf", [P, S])
    if with_mask:
        inp("maskT", [S, W], F32)
    t["outT"] = nc.dram_tensor("outT", [HID, W], F32,
                               kind="ExternalOutput").ap()

    with tile.TileContext(nc) as tc:
        _emit(tc, t, with_mask)
    nc.compile()
    return nc


_PROG_CACHE = {}


def _get_program(with_mask):
    if with_mask not in _PROG_CACHE:
        _PROG_CACHE[with_mask] = _build_program(with_mask)
    return _PROG_CACHE[with_mask]


def _block4(w, mb, kb):
    """[kb*P, mb*P] -> [mb, P, kb, P] with W[m,p,k,c] = w[k*P+p, m*P+c]."""
    return np.ascontiguousarray(
        w.reshape(kb, P, mb, P).transpose(2, 1, 0, 3))


def make_in_maps(hidden_states, attention_mask, cos, sin, w_qa, qa_ln, w_qb,
                 w_kva, kva_ln, w_kvb, w_o, with_mask):
    f32 = np.float32
    c = np.ascontiguousarray

    w_qb_r = np.asarray(w_qb, f32).reshape(QLR, NH, DQK)
    w_qb_re = np.concatenate(
        [w_qb_r[:, :, :DN].reshape(QLR, NH * DN),
         w_qb_r[:, :, DN:].reshape(QLR, NH * DR)], axis=1)
    w_kva_pad = np.concatenate(
        [np.asarray(w_kva, f32), np.zeros((HID, P - DR), f32)], axis=1)
    kvb = np.asarray(w_kvb, f32).reshape(KB_CKV, P, NH, DN + DV)
    w_kvb_kn = c(kvb[:, :, :, :DN].transpose(2, 1, 0, 3)
                 .astype(NPBF))                        # [NH, P, KB_CKV, DN]
    w_kvb_v = c(kvb[:, :, :, DN:].reshape(KB_CKV, P, NH // 2, 2 * DV)
                .transpose(2, 1, 0, 3).astype(NPBF))   # [NH/2, P, kc, 256]

    qa_ln_p = c(np.asarray(qa_ln, f32).reshape(MB_QLR, P).T)
    kva_ln_p = c(np.asarray(kva_ln, f32).reshape(KB_CKV, P).T)

    cosT = np.asarray(cos, f32).T                      # [64, S]
    sinT = np.asarray(sin, f32).T
    sin_s = np.concatenate([-sinT[:DR // 2], sinT[DR // 2:]], axis=0)
    cos2 = c(np.concatenate([cosT, cosT], axis=0))     # [128, S]
    sin2s = c(np.concatenate([sin_s, sin_s], axis=0))

    shared = {
        "w_qa": _block4(np.asarray(w_qa, f32), MB_QLR, KB_HID).astype(NPBF),
        "w_qb": _block4(w_qb_re, MB_NOPE + MB_PE, KB_QLR).astype(NPBF),
        "w_kva": c(w_kva_pad.reshape(KB_HID, P, MB_KVA, P)
                   .transpose(1, 2, 0, 3).astype(NPBF)),
        "w_kvb_kn": w_kvb_kn,
        "w_kvb_v": w_kvb_v,
        "w_o": _block4(np.asarray(w_o, f32), MB_HID, KB_HID).astype(NPBF),
        "qa_ln_p": qa_ln_p,
        "kva_ln_p": kva_ln_p,
    }

    hs = np.asarray(hidden_states)
    am = np.asarray(attention_mask)
    in_maps = []
    for core in range(NCORES):
        b, pnl = divmod(core, NPANEL)
        q0 = pnl * W
        # rotate the key axis so this core's query panel is chunk 0;
        # softmax over keys is permutation-invariant as long as the
        # rope tables (and mask) are rotated identically
        hsT = np.roll(np.asarray(hs[b], f32).T, -q0, axis=1)   # [HID, S]
        hs_pks = c(hsT.reshape(KB_HID, P, S).transpose(1, 0, 2)
                   .astype(NPBF))                      # [128, 16, S]
        m = dict(shared)
        m["hs_pks"] = hs_pks
        m["cos2f"] = c(np.roll(cos2, -q0, axis=1).astype(NPBF))
        m["sin2sf"] = c(np.roll(sin2s, -q0, axis=1).astype(NPBF))
        if with_mask:
            m["maskT"] = c(np.roll(am[b, 0, q0:q0 + W, :].T.astype(f32),
                                   -q0, axis=0))
        in_maps.append(m)
    return in_maps


def kernel(hidden_states, attention_mask, cos, sin, w_qa, qa_ln, w_qb,
           w_kva, kva_ln, w_kvb, w_o):
    global LAST_RESULT
    with_mask = bool(np.any(np.asarray(attention_mask) != 0))
    nc = _get_program(with_mask)
    in_maps = make_in_maps(hidden_states, attention_mask, cos, sin, w_qa,
                           qa_ln, w_qb, w_kva, kva_ln, w_kvb, w_o, with_mask)
    trace = os.environ.get("KERNEL_TRACE", "0") == "1"
    res = bass_utils.run_bass_kernel_spmd(
        nc, in_maps, core_ids=list(range(NCORES)), trace=trace)
    LAST_RESULT = res

    out = np.empty((B, S, HID), np.float32)
    for core in range(NCORES):
        b, pnl = divmod(core, NPANEL)
        q0 = pnl * W
        out[b, q0:q0 + W, :] = res.results[core]["outT"].T
    return out
